# revision 1
# baseline (speedup 1.0000x reference)
"""Trainium2 Bass kernel for nn_MultiHeadedAttention_25984552141341.

Computation (reference):
    q/k/v = (x @ W + b) split into 8 heads of 64
    scores = q @ k^T / 8
    scores += sf_net(scores)   (SoftmaxResNet over the key dim, 71)
    p = softmax(scores, axis=key)
    out = (p @ v merged) @ Wo + bo

Sharding: batch (512) split across 8 NeuronCores, 64 batches each.
All weights replicated. Each core runs an identical Bass program (SPMD).

Device-side layout strategy (per core):
  * activations live feature-on-partition / token-on-free ("transposed"
    layout, xT = [D, B*L]); the host passes query/key/value pre-transposed
    so no on-chip transposes of the big inputs are needed.
  * qT, kT produced as [dout, tok] (transposed) by matmuls with the weight
    tiles as the stationary operand; v produced per-batch in natural
    [tok, dout] layout (needed as the moving operand of p @ v).
  * scores are computed transposed: S' = [k, q] = kT_h.T @ qT_h, which is
    exactly the layout the sf-net matmuls want (contraction over k).
  * sf-net: h1T = sf_w1.T @ S' ([ff, q]); gelu(+b1) on scalar engine;
    h2 accumulated directly back into the scores PSUM region via a second
    matmul; exp(+b2) on scalar engine (softmax without max-subtraction:
    scores2 is bounded by ~3 for this model family).
  * attention: one matmul per head with stationary E'_h = exp(scores2)
    and moving [v_h | 1] producing [q, dh] plus the softmax denominator;
    normalization by per-partition (per-q) reciprocal on vector engine.
  * attn rows are transposed back per batch with the PE transpose and
    assembled into attnT [D, tok] feeding the output projection.
  * biases: bq/bk folded into the PSUM->SBUF copies (scalar engine bias),
    bv folded into the output bias (softmax rows sum to 1, so
    p @ (v + 1 bv^T) = p @ v + 1 bv^T), bo_eff = bo + bv @ Wo computed
    on device once.

All matmuls run in bf16 (fp32 PSUM accumulation); measured end-to-end
scale-relative error vs the fp32 reference ~4e-3.
"""

import contextlib
import os

import numpy as np

import concourse.bass as bass
import concourse.mybir as mybir
import concourse.tile as tile
from concourse import bacc
from concourse import bass_utils
from concourse.masks import make_identity

F32 = mybir.dt.float32
BF16 = mybir.dt.bfloat16
AF = mybir.ActivationFunctionType

N_CORES = 8
B, L, D, H = 512, 71, 512, 8
DH = D // H  # 64
FF = 128  # sf_net hidden
BC = B // N_CORES  # 64 batches per core
T = BC * L  # 4544 tokens per core
GB = 8  # batches per group
G = BC // GB  # 8 groups
GT = GB * L  # 568 tokens per group
HALF = GT // 2  # 284

_CACHE = {}


def _build():
    stage = int(os.environ.get("KSTAGE", "99"))
    nc = bacc.Bacc("TRN2", target_bir_lowering=False, debug=False,
                   num_devices=N_CORES)

    xqT = nc.dram_tensor("xqT", [D, T], F32, kind="ExternalInput").ap()
    xkT = nc.dram_tensor("xkT", [D, T], F32, kind="ExternalInput").ap()
    xvT = nc.dram_tensor("xvT", [D, T], F32, kind="ExternalInput").ap()
    Wq = nc.dram_tensor("Wq", [D, D], F32, kind="ExternalInput").ap()
    Wk = nc.dram_tensor("Wk", [D, D], F32, kind="ExternalInput").ap()
    Wv = nc.dram_tensor("Wv", [D, D], F32, kind="ExternalInput").ap()
    Wo = nc.dram_tensor("Wo", [D, D], F32, kind="ExternalInput").ap()
    bq = nc.dram_tensor("bq", [D], F32, kind="ExternalInput").ap()
    bk = nc.dram_tensor("bk", [D], F32, kind="ExternalInput").ap()
    bv = nc.dram_tensor("bv", [D], F32, kind="ExternalInput").ap()
    bo = nc.dram_tensor("bo", [D], F32, kind="ExternalInput").ap()
    w1 = nc.dram_tensor("sf_w1", [L, FF], F32, kind="ExternalInput").ap()
    b1 = nc.dram_tensor("sf_b1", [FF], F32, kind="ExternalInput").ap()
    w2 = nc.dram_tensor("sf_w2", [FF, L], F32, kind="ExternalInput").ap()
    b2 = nc.dram_tensor("sf_b2", [L], F32, kind="ExternalInput").ap()
    out_d = nc.dram_tensor("out", [T, D], F32, kind="ExternalOutput").ap()

    with tile.TileContext(nc) as tc, contextlib.ExitStack() as ctx:
        singles = ctx.enter_context(tc.tile_pool(name="singles", bufs=1))
        p_xt = ctx.enter_context(tc.tile_pool(name="xt", bufs=2))
        p_qk = ctx.enter_context(tc.tile_pool(name="qk", bufs=2))
        p_v = ctx.enter_context(tc.tile_pool(name="v", bufs=4))
        p_ssb = ctx.enter_context(tc.tile_pool(name="ssb", bufs=3))
        p_s2 = ctx.enter_context(tc.tile_pool(name="s2", bufs=3))
        p_esb = ctx.enter_context(tc.tile_pool(name="esb", bufs=3))
        p_gel = ctx.enter_context(tc.tile_pool(name="gel", bufs=2))
        p_asc = ctx.enter_context(tc.tile_pool(name="asc", bufs=2))
        p_osb = ctx.enter_context(tc.tile_pool(name="osb", bufs=3))
        p_small = ctx.enter_context(tc.tile_pool(name="small", bufs=4))
        # PSUM: 8 banks total.  pp: 2x1 bank, sc: 2x2 banks, h1: 1x2 banks.
        ps_pp = ctx.enter_context(tc.tile_pool(name="pp", bufs=2, space="PSUM"))
        ps_sc = ctx.enter_context(tc.tile_pool(name="sc", bufs=2, space="PSUM"))
        ps_h1 = ctx.enter_context(tc.tile_pool(name="h1", bufs=1, space="PSUM"))

        # ---- constants / weights ----
        def w_tiles(w_ap, name):
            t = singles.tile([128, 4, D], BF16, tag=f"w_{name}")
            nc.gpsimd.dma_start(out=t, in_=w_ap.rearrange("(j p) d -> p j d", p=128))
            return t

        Wq_sb = w_tiles(Wq, "q")
        Wk_sb = w_tiles(Wk, "k")
        Wv_sb = w_tiles(Wv, "v")
        Wo_sb = w_tiles(Wo, "o")

        def b_tile(b_ap, name):
            t = singles.tile([128, 4], F32, tag=f"b_{name}")
            nc.gpsimd.dma_start(out=t, in_=b_ap.rearrange("(j p) -> p j", p=128))
            return t

        bq_sb = b_tile(bq, "q")
        bk_sb = b_tile(bk, "k")
        bq8_sb = singles.tile([128, 4], F32, tag="bq8")
        nc.scalar.mul(bq8_sb, bq_sb, 0.125)

        w1_sb = singles.tile([L, FF], BF16, tag="w1")
        nc.gpsimd.dma_start(out=w1_sb, in_=w1)
        w2_sb = singles.tile([FF, L], BF16, tag="w2")
        nc.gpsimd.dma_start(out=w2_sb, in_=w2)
        b1_sb = singles.tile([FF, 1], F32, tag="b1")
        nc.gpsimd.dma_start(out=b1_sb, in_=b1.rearrange("(p o) -> p o", o=1))
        b2_sb = singles.tile([L, 1], F32, tag="b2")
        nc.gpsimd.dma_start(out=b2_sb, in_=b2.rearrange("(p o) -> p o", o=1))

        ident = singles.tile([L, L], BF16, tag="ident")
        make_identity(nc, ident)

        # bo_eff = bo + bv @ Wo, replicated to [128, D]
        bv_sb = singles.tile([128, 4], BF16, tag="bv")
        nc.gpsimd.dma_start(out=bv_sb, in_=bv.rearrange("(j p) -> p j", p=128))
        bo_sb = singles.tile([1, D], F32, tag="bo")
        nc.gpsimd.dma_start(out=bo_sb, in_=bo.rearrange("(o d) -> o d", o=1))
        ps_bvwo = ps_pp.tile([1, D], F32, tag="pp")
        for j in range(4):
            nc.tensor.matmul(ps_bvwo, bv_sb[:, j:j + 1], Wo_sb[:, j, :],
                             start=(j == 0), stop=(j == 3))
        boeff_row = singles.tile([1, D], F32, tag="boeffrow")
        nc.vector.tensor_add(boeff_row, ps_bvwo, bo_sb)
        ones_f32 = singles.tile([1, 128], F32, tag="ones1")
        nc.vector.memset(ones_f32, 1.0)
        ps_rep = ps_pp.tile([128, D], F32, tag="pp")
        nc.tensor.matmul(ps_rep, ones_f32, boeff_row, start=True, stop=True)
        BO_sb = singles.tile([128, D], F32, tag="BO")
        nc.vector.tensor_copy(out=BO_sb, in_=ps_rep)

        attnT = singles.tile([128, 4, T], BF16, tag="attnT")

        xq3 = xqT.rearrange("(j p) t -> p j t", p=128)
        xk3 = xkT.rearrange("(j p) t -> p j t", p=128)
        xv3 = xvT.rearrange("(j p) t -> p j t", p=128)

        for g in range(G):
            if stage < 1:
                break
            t0 = g * GT
            xtq = p_xt.tile([128, 4, GT], BF16, tag="xtq")
            xtk = p_xt.tile([128, 4, GT], BF16, tag="xtk")
            xtv = p_xt.tile([128, 4, GT], BF16, tag="xtv")
            nc.gpsimd.dma_start(out=xtq, in_=xq3[:, :, t0:t0 + GT])
            nc.gpsimd.dma_start(out=xtk, in_=xk3[:, :, t0:t0 + GT])
            nc.gpsimd.dma_start(out=xtv, in_=xv3[:, :, t0:t0 + GT])

            # --- q/k projections (transposed layout [dout, tok]) ---
            qT = p_qk.tile([128, 4, GT], BF16, tag="qT")
            kT = p_qk.tile([128, 4, GT], BF16, tag="kT")
            for dt_ in range(4):
                for hf in range(2):
                    c0 = hf * HALF
                    pq = ps_pp.tile([128, HALF], F32, tag="pp")
                    for j in range(4):
                        nc.tensor.matmul(
                            pq, Wq_sb[:, j, dt_ * 128:(dt_ + 1) * 128],
                            xtq[:, j, c0:c0 + HALF],
                            start=(j == 0), stop=(j == 3))
                    nc.scalar.activation(
                        out=qT[:, dt_, c0:c0 + HALF], in_=pq, func=AF.Identity,
                        bias=bq8_sb[:, dt_:dt_ + 1], scale=0.125)
                    pk = ps_pp.tile([128, HALF], F32, tag="pp")
                    for j in range(4):
                        nc.tensor.matmul(
                            pk, Wk_sb[:, j, dt_ * 128:(dt_ + 1) * 128],
                            xtk[:, j, c0:c0 + HALF],
                            start=(j == 0), stop=(j == 3))
                    nc.vector.tensor_scalar_add(
                        out=kT[:, dt_, c0:c0 + HALF], in0=pk,
                        scalar1=bk_sb[:, dt_:dt_ + 1])

            # Batches are processed in pairs with the two sf-net phases
            # interleaved so the scalar engine runs [gelu, gelu] then
            # [exp, exp] — gelu and exp live in different ACT LUT sets, so
            # every set switch costs a ~1.3us table reload; pairing halves
            # the reload count.  The v projections are emitted one pair
            # ahead, between that pair's h2 matmuls and its attention
            # matmuls, giving the PE ~1.7us of independent work to chew on
            # while the scalar engine reloads the exp LUT and evaluates it.
            vq = {}

            def emit_vproj(bl):
                tc0 = bl * L
                pv = ps_pp.tile([L, D], F32, tag="pp")
                for j in range(4):
                    nc.tensor.matmul(pv, xtv[:, j, tc0:tc0 + L],
                                     Wv_sb[:, j, :],
                                     start=(j == 0), stop=(j == 3))
                v_sb = p_v.tile([L, H, DH + 1], BF16, tag="v")
                nc.gpsimd.memset(v_sb[:, :, DH:DH + 1], 1.0)
                nc.vector.tensor_copy(out=v_sb[:, :, 0:DH],
                                      in_=pv.rearrange("p (h d) -> p h d", h=H))
                vq[bl] = v_sb

            for pb in range(GB // 2):
                if stage < 3:
                    break
                pair = (2 * pb, 2 * pb + 1)
                st = {}
                if pb == 0:
                    for bl in pair:
                        emit_vproj(bl)

                for bl in pair:
                    tc0 = bl * L
                    v_sb = vq.pop(bl)

                    # --- scores S' = [k, q] ---
                    # PE row groups must NOT alternate between matmuls (HW
                    # wedge observed when the base partition flips 0<->64
                    # inside a bank group), so heads are emitted parity-
                    # grouped: bank 0 hosts even heads (qkT partition base
                    # 0), bank 1 odd heads (base 64).  Head h lives at
                    # column 512*(h%2) + 71*(h//2).
                    S_ps = ps_sc.tile([L, 1024], F32, tag="sc")
                    for i in range(2):
                        for hh in range(4):  # head 2*hh+i
                            off = 512 * i + L * hh
                            nc.tensor.matmul(
                                S_ps[:, off:off + L],
                                kT[64 * i:64 * i + 64, hh, tc0:tc0 + L],
                                qT[64 * i:64 * i + 64, hh, tc0:tc0 + L],
                                start=(hh == 0), stop=False)
                    S3 = S_ps.rearrange("p (b r) -> p b r", b=2)[:, :, 0:4 * L]
                    Ssb = p_ssb.tile([L, 2, 4 * L], BF16, tag="Ssb")
                    nc.vector.tensor_copy(out=Ssb, in_=S3)
                    Sflat = Ssb.rearrange("p b r -> p (b r)")

                    # --- sf-net hidden layer ---
                    h1_ps = ps_h1.tile([FF, GT], F32, tag="h1")
                    nc.tensor.matmul(h1_ps[:, 0:512], w1_sb, Sflat[:, 0:512],
                                     start=True, stop=True)
                    nc.tensor.matmul(h1_ps[:, 512:GT], w1_sb, Sflat[:, 512:GT],
                                     start=True, stop=True)
                    gel = p_gel.tile([FF, GT], BF16, tag="gel")
                    nc.scalar.activation(out=gel, in_=h1_ps, func=AF.Gelu,
                                         bias=b1_sb, scale=1.0)
                    st[bl] = (S_ps, S3, gel, v_sb)

                Ef = {}
                for bl in pair:
                    S_ps, S3, gel, v_sb = st[bl]
                    # --- sf-net output accumulated onto the scores ---
                    for hf in range(2):
                        nc.tensor.matmul(
                            S_ps[:, 512 * hf:512 * hf + 4 * L], w2_sb,
                            gel[:, 4 * L * hf:4 * L * (hf + 1)],
                            start=False, stop=True)
                    # --- softmax (no max subtraction; |scores2| < ~4) ---
                    E_sb = p_esb.tile([L, 2, 4 * L], BF16, tag="E")
                    nc.scalar.activation(out=E_sb, in_=S3, func=AF.Exp,
                                         bias=b2_sb, scale=1.0)
                    Ef[bl] = E_sb.rearrange("p b r -> p (b r)")

                # v projections for the NEXT pair: independent PE work that
                # hides the exp LUT reload + evaluation latency.
                if pb + 1 < GB // 2:
                    for bl in (2 * pb + 2, 2 * pb + 3):
                        emit_vproj(bl)

                for bl in pair:
                    bi = g * GB + bl
                    S_ps, S3, gel, v_sb = st[bl]
                    Eflat = Ef[bl]
                    # --- attention + denominators ---
                    # E/pa column block p hosts head h = 2*(p%4) + p//4 (the
                    # parity-grouped order from the scores layout).
                    pa = ps_h1.tile([L, 1024], F32, tag="h1")
                    for p in range(H):
                        h = 2 * (p % 4) + (p // 4)
                        off = 512 * (p // 4) + (DH + 1) * (p % 4)
                        nc.tensor.matmul(
                            pa[:, off:off + DH + 1],
                            Eflat[:, L * p:L * p + L], v_sb[:, h, :],
                            start=(p % 4 == 0), stop=(p % 4 == 3))
                    recip = p_small.tile([L, 2, 4], F32, tag="recip")
                    pa4 = pa.rearrange("p (b r) -> p b r", b=2)
                    for bnk in range(2):
                        den = pa4[:, bnk, 0:4 * (DH + 1)].rearrange(
                            "p (h c) -> p h c", h=4)[:, :, DH:DH + 1]
                        nc.vector.reciprocal(out=recip[:, bnk, :], in_=den)
                    # scale + cast; scatter bank b's blocks (heads 2*hh+b)
                    # to their true positions: col 128*hh + 64*b
                    asc = p_asc.tile([L, D], BF16, tag="asc")
                    for bnk in range(2):
                        nc.vector.tensor_mul(
                            bass.AP(tensor=asc.tensor,
                                    offset=asc.offset + DH * bnk,
                                    ap=[asc.ap[0], [2 * DH, 4], [1, DH]]),
                            pa4[:, bnk, 0:4 * (DH + 1)].rearrange(
                                "p (h c) -> p h c", h=4)[:, :, 0:DH],
                            bass.AP(tensor=recip.tensor,
                                    offset=recip.offset + 4 * bnk,
                                    ap=[recip.ap[0], [1, 4], [0, DH]]))

                    # --- transpose attn rows to [feat, tok] and assemble ---
                    tp = ps_pp.tile([128, 4, L + 1], BF16, tag="pp")
                    for j in range(4):
                        nc.tensor.transpose(tp[:, j, 0:L],
                                            asc[:, 128 * j:128 * (j + 1)],
                                            ident)
                    nc.vector.tensor_copy(
                        out=attnT[:, :, bi * L:(bi + 1) * L],
                        in_=tp[:, :, 0:L])

        # --- output projection out = attnT.T @ Wo + bo_eff ---
        n_chunks = (T + 127) // 128 if stage >= 10 else 0
        for c in range(n_chunks):
            w = min(128, T - c * 128)
            po = ps_pp.tile([128, D], F32, tag="pp")
            for j in range(4):
                nc.tensor.matmul(po[0:w], attnT[:, j, c * 128:c * 128 + w],
                                 Wo_sb[:, j, :], start=(j == 0), stop=(j == 3))
            osb = p_osb.tile([128, D], F32, tag="osb")
            nc.vector.tensor_add(osb[0:w], po[0:w], BO_sb[0:w])
            nc.sync.dma_start(out=out_d[c * 128:c * 128 + w, :], in_=osb[0:w])

    nc.compile()
    return nc


def _get_nc():
    if "nc" not in _CACHE:
        _CACHE["nc"] = _build()
    return _CACHE["nc"]


def _prep_in_maps(inputs):
    f32 = lambda a: np.ascontiguousarray(np.asarray(a, dtype=np.float32))
    shared = {k: f32(inputs[k]) for k in
              ("Wq", "Wk", "Wv", "Wo", "bq", "bk", "bv", "bo",
               "sf_w1", "sf_b1", "sf_w2", "sf_b2")}
    xT = {}
    for key, name in (("query", "xqT"), ("key", "xkT"), ("value", "xvT")):
        # [B, L, D] -> [D, B, L], feature-major (layout change only)
        xT[name] = np.asarray(inputs[key], dtype=np.float32).transpose(2, 0, 1)
    in_maps = []
    for c in range(N_CORES):
        m = dict(shared)
        for name in ("xqT", "xkT", "xvT"):
            m[name] = np.ascontiguousarray(
                xT[name][:, c * BC:(c + 1) * BC, :]).reshape(D, T)
        in_maps.append(m)
    return in_maps


def run(inputs, trace=False):
    nc = _get_nc()
    in_maps = _prep_in_maps(inputs)
    res = bass_utils.run_bass_kernel_spmd(
        nc, in_maps, core_ids=list(range(N_CORES)), trace=trace)
    out = np.concatenate(
        [res.results[c]["out"].reshape(BC, L, D) for c in range(N_CORES)],
        axis=0)
    return out, res


def kernel(**inputs) -> np.ndarray:
    out, _ = run(inputs, trace=False)
    return out

